# revision 1
# baseline (speedup 1.0000x reference)
"""DiT attention block (qkv -> qk-rmsnorm -> rope -> sdpa -> proj) on 8 trn2
NeuronCores.

Sharding: core c = (batch b=c//4, query-chunk qc=c%4). Each core computes the
full K/V for its batch (replicated across the 4 cores of the batch -- the
pre-head-split RMS norm forces full-width QKV anyway), and attention + output
projection for its 512 query rows. No cross-core communication; the host
scatters inputs and gathers the 8 output shards.

Precision: QKV/proj matmuls run in float32r (TF32-like, 1 cyc/row vs 4 for
fp32) on host-pre-truncated operands. The attention side (kT/qT/P/V) is fp16
(10-bit mantissa; transposes run 2x faster and SBUF halves; measured output
error stays ~1e-3 relative to output absmax). Attention uses an S^T layout
(scores [k, q]) so softmax needs no transposes: exp runs on ScalarE straight
out of PSUM (logits are O(+-10), no max subtraction) and denominators come
from a ones-column appended to V. Head pairs run their K=64 score matmuls
concurrently in the upper/lower halves of the PE array via tile_position.
"""

import sys

for _p in ("/opt/trn_rl_repo", "/root/.axon_site/_ro/trn_rl_repo"):
    if _p not in sys.path:
        sys.path.append(_p)

import numpy as np

N_CORES = 8
B, N, D = 2, 2048, 1024
H, HD = 16, 64
NQ = 512          # query rows per core
EPS = 1e-6
NCH = N // 128    # 16 seq chunks per batch
NQC = NQ // 128   # 4 query chunks per core
ND = D // 128     # 8 contraction chunks

_MODULE_CACHE = {}


def _trunc_r(a):
    """Round fp32 -> fp32r domain (zero low 12 mantissa bits). Pre-truncated
    values are fixed points of the HW rounding, so DMA-ing them into float32r
    tiles is exact."""
    a = np.ascontiguousarray(a, dtype=np.float32)
    return (a.view(np.uint32) & np.uint32(0xFFFFF000)).view(np.float32)


def build_module(zero_bqkv=True, unit_qscale=True, unit_kscale=True, zero_bout=True):
    key = (zero_bqkv, unit_qscale, unit_kscale, zero_bout)
    if key in _MODULE_CACHE:
        return _MODULE_CACHE[key]

    import concourse.bacc as bacc
    import concourse.mybir as mybir
    import concourse.tile as tile

    F32, F32R, F16 = mybir.dt.float32, mybir.dt.float32r, mybir.dt.float16
    AF = mybir.ActivationFunctionType

    nc = bacc.Bacc("TRN2", target_bir_lowering=False, debug=False,
                   num_devices=N_CORES)

    xT = nc.dram_tensor("xT", [D, N], F32R, kind="ExternalInput").ap()
    xqT = nc.dram_tensor("xqT", [D, NQ], F32R, kind="ExternalInput").ap()
    w = nc.dram_tensor("w", [D, 3 * D], F32R, kind="ExternalInput").ap()
    wo = nc.dram_tensor("wo", [D, D], F32R, kind="ExternalInput").ap()
    csk = nc.dram_tensor("csk", [N, HD], F32, kind="ExternalInput").ap()
    csq = nc.dram_tensor("csq", [NQ, HD], F32, kind="ExternalInput").ap()
    ident_d = nc.dram_tensor("ident", [128, 128], F16, kind="ExternalInput").ap()
    if not unit_kscale:
        kscale_d = nc.dram_tensor("kscale", [1, D], F32, kind="ExternalInput").ap()
    if not unit_qscale:
        qscale_d = nc.dram_tensor("qscale", [1, D], F32, kind="ExternalInput").ap()
    if not zero_bqkv:
        bqkv_d = nc.dram_tensor("bqkv", [1, 3 * D], F32, kind="ExternalInput").ap()
    if not zero_bout:
        bout_d = nc.dram_tensor("bout", [1, D], F32, kind="ExternalInput").ap()
    out_d = nc.dram_tensor("out", [NQ, D], F32, kind="ExternalOutput").ap()
    vscr = nc.dram_tensor("vscr", [H, 128, NCH, HD + 1], F16,
                          kind="Internal").ap()

    with tile.TileContext(nc) as tc:
        with tc.tile_pool(name="persist", bufs=1) as pp:
            # kT/qT stored flat in fp16: channel-chunk j at free offset j*N
            kT = pp.tile([128, ND * N], F16, name="kT")
            qT = pp.tile([128, ND * NQ], F16, name="qT")
            ident = pp.tile([128, 128], F16, name="ident_t")
            nc.sync.dma_start(out=ident[:, :], in_=ident_d[:, :])
            epsb = pp.tile([128, 1], F32, name="epsb")
            nc.vector.memset(epsb[:, :], EPS)
            ones16 = pp.tile([128, 16], F16, name="ones16")
            nc.vector.memset(ones16[:, :], 1.0)
            if not unit_kscale:
                ksc_b = pp.tile([128, D], F32, name="ksc_b")
                nc.sync.dma_start(out=ksc_b[:, :],
                                  in_=kscale_d[0:1, :].broadcast_to([128, D]))
            if not unit_qscale:
                qsc_b = pp.tile([128, D], F32, name="qsc_b")
                nc.sync.dma_start(out=qsc_b[:, :],
                                  in_=qscale_d[0:1, :].broadcast_to([128, D]))
            if not zero_bqkv:
                bq_b = pp.tile([128, 3 * D], F32, name="bq_b")
                nc.sync.dma_start(out=bq_b[:, :],
                                  in_=bqkv_d[0:1, :].broadcast_to([128, 3 * D]))
            if not zero_bout:
                bo_b = pp.tile([128, D], F32, name="bo_b")
                nc.sync.dma_start(out=bo_b[:, :],
                                  in_=bout_d[0:1, :].broadcast_to([128, D]))

            # ---- shared emitters -------------------------------------------
            def emit_block_matmuls(psum, xt, wtiles, col0):
                """psum[128,1024] += xT_chunk.T @ W[:, col0:col0+1024]"""
                for half in range(2):
                    o = psum[:, half * 512:(half + 1) * 512]
                    for d in range(ND):
                        nc.tensor.matmul(
                            o, xt[d],
                            wtiles[d][:, col0 + half * 512: col0 + (half + 1) * 512],
                            start=(d == 0), stop=(d == ND - 1))

            def load_xt(work, src, ci):
                xt = work.tile([128, ND * 128], F32R, tag="xt", name="xt",
                               bufs=3)
                src4 = src.rearrange("(d p) n -> p d n", p=128)
                nc.sync.dma_start(
                    out=xt.rearrange("p (d j) -> p d j", j=128)[:, :, :],
                    in_=src4[:, :, ci * 128:(ci + 1) * 128])
                return [xt[:, d * 128:(d + 1) * 128] for d in range(ND)]

            def emit_norm_rope(work, psum, ci, cs_d, scale_tile, bias_slice):
                """RMS-norm + scale + rope the natural [128, 1024] rows in
                `psum` -> fp16 kr SBUF tile (returned)."""
                if bias_slice is not None:
                    kb = work.tile([128, D], F32, tag="kb", name="kb")
                    nc.vector.tensor_add(kb[:, :], psum[:, :], bias_slice[:, :])
                    src = kb
                else:
                    src = psum
                ksq = work.tile([128, D], F32, tag="ksq", name="ksq", bufs=1)
                ssq = work.tile([128, 1], F32, tag="ssq", name="ssq")
                nc.scalar.activation(ksq[:, :], src[:, :], AF.Square,
                                     accum_out=ssq[:, :])
                rms = work.tile([128, 1], F32, tag="rms", name="rms")
                nc.scalar.activation(rms[:, :], ssq[:, :], AF.Sqrt,
                                     scale=1.0 / D, bias=epsb[:, :])
                rstd = work.tile([128, 1], F32, tag="rstd", name="rstd")
                nc.vector.reciprocal(rstd[:, :], rms[:, :])
                if scale_tile is not None:
                    kbs = work.tile([128, D], F32, tag="kbs", name="kbs")
                    nc.vector.tensor_mul(kbs[:, :], src[:, :], scale_tile[:, :])
                    src = kbs
                cssm = work.tile([128, HD], F32, tag="cssm", name="cssm")
                nc.sync.dma_start(out=cssm[:, :],
                                  in_=cs_d[ci * 128:(ci + 1) * 128, :])
                cst = work.tile([128, H * HD], F32, tag="cst", name="cst")
                cst3 = cst.rearrange("p (h w) -> p h w", w=HD)
                # broadcast across heads + fold the rmsnorm 1/rms, in one op
                nc.vector.tensor_scalar_mul(
                    cst3[:, :, :],
                    cssm[:, None, :].broadcast_to([128, H, HD]),
                    rstd[:, :])
                crs = cst3[:, :, 0:HD // 2]
                srs = cst3[:, :, HD // 2:HD]
                kr = work.tile([128, D], F16, tag="kr", name="kr", bufs=3)
                t1 = work.tile([128, 512], F32, tag="t1", name="t1")
                t2 = work.tile([128, 512], F32, tag="t2", name="t2")
                src3 = src.rearrange("p (h j two) -> p h j two", j=HD // 2, two=2)
                kr3 = kr.rearrange("p (h j two) -> p h j two", j=HD // 2, two=2)
                ke, ko = src3[:, :, :, 0], src3[:, :, :, 1]
                re, rv = kr3[:, :, :, 0], kr3[:, :, :, 1]
                t13 = t1.rearrange("p (h j) -> p h j", j=HD // 2)
                t23 = t2.rearrange("p (h j) -> p h j", j=HD // 2)
                # DVE: 4 muls (read PSUM); GpSimd: sub/add (SBUF-only, fp16 out)
                nc.vector.tensor_mul(t13[:, :, :], ko, srs)
                nc.vector.tensor_mul(re, ke, crs)
                nc.vector.tensor_mul(t23[:, :, :], ke, srs)
                nc.vector.tensor_mul(rv, ko, crs)
                nc.gpsimd.tensor_sub(re, re, t13[:, :, :])
                nc.gpsimd.tensor_add(rv, rv, t23[:, :, :])
                return kr

            def emit_transposes(psT, kr, ci, dstT, dst_stride):
                """kr fp16 [128(seq), 1024(ch)] -> dstT[:, j*dst_stride+ci*128]"""
                tps = psT.tile([128, D], F16, tag="tps", name="tps")
                for j in range(ND):
                    nc.tensor.transpose(tps[:, j * 128:(j + 1) * 128],
                                        kr[:, j * 128:(j + 1) * 128],
                                        ident[:, :])
                dst = dstT.rearrange("p (j n) -> p j n", n=dst_stride)
                nc.scalar.activation(
                    dst[:, :, ci * 128:(ci + 1) * 128],
                    tps.rearrange("p (j n) -> p j n", n=128)[:, :, :],
                    AF.Copy)

            def emit_v_store(work, vps, ci):
                vt = work.tile([128, H * (HD + 1)], F16, tag="vt",
                               name="vt", bufs=3)
                vt3 = vt.rearrange("p (h w) -> p h w", w=HD + 1)
                vps3 = vps.rearrange("p (h w) -> p h w", w=HD)
                if zero_bqkv:
                    nc.scalar.activation(vt3[:, :, 0:HD], vps3[:, :, :],
                                         AF.Copy)
                else:
                    nc.vector.tensor_add(
                        vt3[:, :, 0:HD], vps3[:, :, :],
                        bq_b[:, 2 * D:3 * D]
                        .rearrange("p (h w) -> p h w", w=HD)[:, :, :])
                nc.vector.tensor_copy(vt3[:, :, HD], ones16[:, :])
                nc.gpsimd.dma_start(
                    out=vscr.rearrange("h p kc w -> p h kc w")[:, :, ci, :],
                    in_=vt3[:, :, :])

            # ---- phase A1: K and V ---------------------------------------
            with tc.tile_pool(name="wkv", bufs=1) as wkvp, \
                 tc.tile_pool(name="awk", bufs=2) as aw, \
                 tc.tile_pool(name="psA", bufs=2, space="PSUM") as psA, \
                 tc.tile_pool(name="psV", bufs=1, space="PSUM") as psV, \
                 tc.tile_pool(name="psT", bufs=1, space="PSUM") as psT:
                wkv = [wkvp.tile([128, 2 * D], F32R, name=f"wkv{d}")
                       for d in range(ND)]
                for d in range(ND):
                    nc.scalar.dma_start(out=wkv[d][:, :],
                                        in_=w[d * 128:(d + 1) * 128, D:3 * D])
                stage1 = {}   # ci -> (kps, vps)
                stage2 = {}   # ci -> kr
                for ci in range(NCH + 2):
                    if ci < NCH:
                        xt = load_xt(aw, xT, ci)
                        kps = psA.tile([128, D], F32, tag="kps", name="kps")
                        emit_block_matmuls(kps, xt, wkv, 0)
                        vps = psV.tile([128, D], F32, tag="vps", name="vps")
                        emit_block_matmuls(vps, xt, wkv, D)
                        stage1[ci] = (kps, vps)
                    if ci - 1 in stage1:
                        kps1, vps1 = stage1.pop(ci - 1)
                        stage2[ci - 1] = emit_norm_rope(
                            aw, kps1, ci - 1, csk,
                            None if unit_kscale else ksc_b,
                            None if zero_bqkv else bq_b[:, D:2 * D])
                        emit_v_store(aw, vps1, ci - 1)
                    if ci - 2 in stage2:
                        emit_transposes(psT, stage2.pop(ci - 2), ci - 2, kT, N)

            # ---- phase A2: Q ---------------------------------------------
            with tc.tile_pool(name="wq", bufs=1) as wqp, \
                 tc.tile_pool(name="awq", bufs=2) as aw, \
                 tc.tile_pool(name="psQ", bufs=2, space="PSUM") as psQ, \
                 tc.tile_pool(name="psT2", bufs=2, space="PSUM") as psT2:
                wq = [wqp.tile([128, D], F32R, name=f"wq{d}") for d in range(ND)]
                for d in range(ND):
                    nc.scalar.dma_start(out=wq[d][:, :],
                                        in_=w[d * 128:(d + 1) * 128, 0:D])
                stage1, stage2 = {}, {}
                for ci in range(NQC + 2):
                    if ci < NQC:
                        xt = load_xt(aw, xqT, ci)
                        qps = psQ.tile([128, D], F32, tag="qps", name="qps")
                        emit_block_matmuls(qps, xt, wq, 0)
                        stage1[ci] = qps
                    if ci - 1 in stage1:
                        stage2[ci - 1] = emit_norm_rope(
                            aw, stage1.pop(ci - 1), ci - 1, csq,
                            None if unit_qscale else qsc_b,
                            None if zero_bqkv else bq_b[:, 0:D])
                    if ci - 2 in stage2:
                        emit_transposes(psT2, stage2.pop(ci - 2), ci - 2, qT, NQ)

            # ---- phase B: attention --------------------------------------
            with tc.tile_pool(name="oTp", bufs=1) as oTp, \
                 tc.tile_pool(name="wop", bufs=1) as wop:
                oT = oTp.tile([128, ND * NQ], F32R, name="oT")
                oT3 = oT.rearrange("p (j n) -> p j n", n=NQ)
                wot = [wop.tile([128, D], F32R, name=f"wot{d}") for d in range(ND)]
                for d in range(ND):
                    nc.sync.dma_start(out=wot[d][:, :],
                                      in_=wo[d * 128:(d + 1) * 128, :])
                vsrc = vscr.rearrange("h p kc w -> h p (kc w)")
                kT3 = kT.rearrange("p (j n) -> p j n", n=N)
                qT3 = qT.rearrange("p (j n) -> p j n", n=NQ)
                with tc.tile_pool(name="bwork", bufs=2) as bw, \
                     tc.tile_pool(name="ptp", bufs=3) as ptp, \
                     tc.tile_pool(name="psS", bufs=2, space="PSUM") as psS, \
                     tc.tile_pool(name="psO", bufs=2, space="PSUM") as psO:
                    for hp in range(H // 2):
                        ch = hp
                        vax0 = bw.tile([128, H * (HD + 1)], F16, tag="vax0",
                                       name="vax0")
                        vax1 = bw.tile([128, H * (HD + 1)], F16, tag="vax1",
                                       name="vax1")
                        v0_3 = vax0.rearrange("p (kc w) -> p kc w", w=HD + 1)
                        v1_3 = vax1.rearrange("p (kc w) -> p kc w", w=HD + 1)
                        nc.gpsimd.dma_start(out=vax0[:, :],
                                            in_=vsrc[2 * hp, :, :])
                        nc.gpsimd.dma_start(out=vax1[:, :],
                                            in_=vsrc[2 * hp + 1, :, :])
                        oaps0 = psO.tile([HD + 1, 512], F32, tag="oaps0",
                                         name="oaps0")
                        oaps1 = psO.tile([HD + 1, 512], F32, tag="oaps1",
                                         name="oaps1")
                        pts = []

                        def emit_s(kc):
                            sps = psS.tile([128, 1024], F32, tag="sps",
                                           name="sps")
                            nc.tensor.matmul(
                                sps[:, 0:512],
                                kT3[0:64, ch, kc * 128:(kc + 1) * 128],
                                qT3[0:64, ch, :],
                                start=True, stop=True, tile_position=(0, 0))
                            nc.tensor.matmul(
                                sps[:, 512:1024],
                                kT3[64:128, ch, kc * 128:(kc + 1) * 128],
                                qT3[64:128, ch, :],
                                start=True, stop=True, tile_position=(64, 0))
                            pt = ptp.tile([128, 1024], F16, tag="pt",
                                          name="pt")
                            nc.scalar.activation(pt[:, :], sps[:, :], AF.Exp,
                                                 scale=float(HD) ** -0.5)
                            pts.append(pt)

                        def emit_o(kc):
                            pt = pts[kc]
                            nc.tensor.matmul(
                                oaps0[:, :], v0_3[:, kc, :], pt[:, 0:512],
                                start=(kc == 0), stop=(kc == NCH - 1))
                            nc.tensor.matmul(
                                oaps1[:, :], v1_3[:, kc, :], pt[:, 512:1024],
                                start=(kc == 0), stop=(kc == NCH - 1))

                        for kc in range(NCH):
                            emit_s(kc)
                            if kc >= 1:
                                emit_o(kc - 1)
                        emit_o(NCH - 1)
                        for i, oaps in ((0, oaps0), (1, oaps1)):
                            rowt = bw.tile([1, 512], F32, tag=f"rowt{i}",
                                           name=f"rowt{i}")
                            nc.vector.reciprocal(rowt[0:1, :],
                                                 oaps[HD:HD + 1, :])
                            rb = bw.tile([64, 512], F32, tag=f"rb{i}",
                                         name=f"rb{i}")
                            nc.gpsimd.partition_broadcast(rb[:, :],
                                                          rowt[0:1, :])
                            nc.vector.tensor_mul(
                                oT3[i * 64:(i + 1) * 64, ch, :],
                                oaps[0:HD, :], rb[:, :])

                # ---- phase C: output projection ---------------------------
                with tc.tile_pool(name="cwork", bufs=2) as cw, \
                     tc.tile_pool(name="psC", bufs=2, space="PSUM") as psC:
                    for rc in range(NQC):
                        ops_ = psC.tile([128, D], F32, tag="ops", name="ops")
                        for half in range(2):
                            o = ops_[:, half * 512:(half + 1) * 512]
                            for j in range(ND):
                                nc.tensor.matmul(
                                    o, oT3[:, j, rc * 128:(rc + 1) * 128],
                                    wot[j][:, half * 512:(half + 1) * 512],
                                    start=(j == 0), stop=(j == ND - 1))
                        ob = cw.tile([128, D], F32, tag="ob", name="ob")
                        if zero_bout:
                            nc.scalar.activation(ob[:, :], ops_[:, :], AF.Copy)
                        else:
                            nc.vector.tensor_add(ob[:, :], ops_[:, :],
                                                 bo_b[:, :])
                        nc.sync.dma_start(out=out_d[rc * 128:(rc + 1) * 128, :],
                                          in_=ob[:, :])

    nc.compile()
    _MODULE_CACHE[key] = nc
    return nc


def make_in_maps(x, cos, sin, Wqkv, bqkv, q_scale, k_scale, Wout, bout,
                 zero_bqkv, unit_qscale, unit_kscale, zero_bout):
    w_r = _trunc_r(Wqkv)
    wo_r = _trunc_r(Wout)
    ident = np.eye(128, dtype=np.float16)
    cs = np.ascontiguousarray(
        np.concatenate([np.asarray(cos, np.float32),
                        np.asarray(sin, np.float32)], axis=1))
    in_maps = []
    for c in range(N_CORES):
        b, qc = c // 4, c % 4
        xTb = _trunc_r(np.asarray(x[b], np.float32).T)
        m = {
            "xT": xTb,
            "xqT": np.ascontiguousarray(xTb[:, qc * NQ:(qc + 1) * NQ]),
            "w": w_r,
            "wo": wo_r,
            "csk": cs,
            "csq": np.ascontiguousarray(cs[qc * NQ:(qc + 1) * NQ]),
            "ident": ident,
        }
        if not unit_kscale:
            m["kscale"] = np.ascontiguousarray(k_scale, np.float32).reshape(1, D)
        if not unit_qscale:
            m["qscale"] = np.ascontiguousarray(q_scale, np.float32).reshape(1, D)
        if not zero_bqkv:
            m["bqkv"] = np.ascontiguousarray(bqkv, np.float32).reshape(1, 3 * D)
        if not zero_bout:
            m["bout"] = np.ascontiguousarray(bout, np.float32).reshape(1, D)
        in_maps.append(m)
    return in_maps


def kernel(x, cos, sin, Wqkv, bqkv, q_scale, k_scale, Wout, bout, **_kw):
    from concourse import bass_utils

    x = np.asarray(x)
    flags = (bool(np.all(np.asarray(bqkv) == 0)),
             bool(np.all(np.asarray(q_scale) == 1)),
             bool(np.all(np.asarray(k_scale) == 1)),
             bool(np.all(np.asarray(bout) == 0)))
    nc = build_module(*flags)
    in_maps = make_in_maps(x, cos, sin, Wqkv, bqkv, q_scale, k_scale, Wout,
                           bout, *flags)
    res = bass_utils.run_bass_kernel_spmd(nc, in_maps,
                                          core_ids=list(range(N_CORES)))
    out = np.empty((B, N, D), dtype=np.float32)
    for c in range(N_CORES):
        b, qc = c // 4, c % 4
        out[b, qc * NQ:(qc + 1) * NQ, :] = res.results[c]["out"]
    return out

